# revision 27
# baseline (speedup 1.0000x reference)
"""Trainium2 Bass kernel for nn_LogisticMixture (discretized logistic mixture loss).

Contract: kernel(**inputs) takes FULL unsharded numpy inputs
  x      [128, 32, 32, 256] f32
  value  [128, 32, 32, 3]   f32 (integer pixel values 0..255)
  W_conv [256, 100]         f32
  b_conv [100]              f32
and returns the full [128] f32 output (per-image sum of mixture log-probs).

Strategy: pure data parallelism over batch across 8 NeuronCores (16384
pixels/core).  Matmul identical to the proven baseline (x^T-stationary,
W streamed, 2048-px macro-tiles of 16x128-px subtiles).  The epilogue is
built on a sigmoid-difference identity with a products-not-logs mixture:

  d_{k,c} = sigma(X1) - sigma(X2)        (= discretized logistic prob)
  X1 = p + PK,  X2 = (p - r_t) - MK,     p = A * r_t
  A   = vp2 - 127.5*locs_m  (host-prescaled W columns, vp2 = v-127)
  r_t = 1/(127.5*softplus(s_raw))  via ACT chain exp->ln1p->ln->exp
  PK  = 32768*(v==255), MK = 32768*(v==0): sigma saturates EXACTLY to
  1/0 at +-32768, so boundary pixels reduce to cdf / survival with no
  masking ops.
  sz  = sum_k e^{f0_k} * d_k0*d_k1*d_k2 ,  sf = sum_k e^{f0_k}
  mix_lp = ln(sz) - ln(sf)   <- the ONLY logs, done on the HOST from the
  tiny [128, px/128] sz/sf outputs; the per-(k,c) ln and the mixture exp
  cancel, killing the whole log-sum-exp chain on-chip.

Layouts follow the PSUM's natural (s, c, k) order (s = px-subtile slot,
k innermost) so every PSUM evacuation is a contiguous-write ACT op
(strided writes measured 5x slower).  Masks and coupling q are
k-materialized host-side so all fat DVE ops keep the 2x f16 mode
(0.56 ns/elem measured).  ACT needs exactly two tables: natural_log_exp
for the scale chain, sigmoid for the single sigma op per pair; all
sigmas run after the last extraction so there is ~1 switch.
Engines: PE matmul; ACT activations + PSUM copies; DVE f16 elementwise
+ f32 products; Pool a slice of the f32 sigma-subtract.
"""
import sys
import os

for _p in ("/opt/trn_rl_repo", "/root/.axon_site/_ro/trn_rl_repo"):
    if os.path.isdir(_p) and _p not in sys.path:
        sys.path.append(_p)

import numpy as np
import ml_dtypes

import concourse.bass as bass
import concourse.mybir as mybir
import concourse.tile as tile
from concourse import bacc
from concourse.bass_utils import run_bass_kernel_spmd
import concourse.hw_specs as hw_specs

F32 = mybir.dt.float32
F16 = mybir.dt.float16
BF16 = mybir.dt.bfloat16
AL = mybir.AluOpType
AF = mybir.ActivationFunctionType
AX = mybir.AxisListType

N_CORES = 8
X_FP8 = True     # xT in float8e4 (e4m3): halves the dominant DMA stream
D = 256
M = 100          # NUM_MIX * NUM_OUT
K = 10           # mixtures
C = 3            # channels
NS = 16          # 128-px subtiles per macro-tile
KBIG = 32768.0   # mask shift; sigma(+-32768) saturates exactly


def _force_sigma_tables():
    """Keep only sigmoid_and_others + natural_log_exp_and_others non-empty
    so the act-table chooser never picks a set lacking our functions; dict
    order is preserved so set ids stay aligned with act_info.json."""
    if getattr(hw_specs, "_ant_sigma_tables", False):
        return
    orig = hw_specs.get_activation_tables
    import functools

    @functools.cache
    def patched(arch):
        tabs = dict(orig(arch))
        keep = {"sigmoid_and_others", "natural_log_exp_and_others"}
        return {k: (set(v) if k in keep else set()) for k, v in tabs.items()}

    hw_specs.get_activation_tables = patched
    bacc.get_activation_tables = patched
    hw_specs._ant_sigma_tables = True


def _v(ap0, offset, pattern):
    """AP on a tile's [:, :] AP: keep partition dim, replace free dims
    (stride-0 broadcast dims allowed on non-innermost positions)."""
    if not isinstance(ap0, bass.AP):
        ap0 = ap0[:, :]
    return bass.AP(tensor=ap0.tensor, offset=ap0.offset + offset,
                   ap=[list(ap0.ap[0])] + [list(p) for p in pattern])


def build_program(pix=16384, with_bias=False):
    """Single-core SPMD program. pix must be a multiple of 4096."""
    TP = 2048                  # pixels per macro-tile (2 images)
    NT = pix // TP             # macro-tiles
    NP = NT // 2               # 2-tile pairs for the fat epilogue ops
    NKC = NS * K * C           # 480 per tile
    PW = 2 * NKC               # 960 per pair
    NK = NS * K                # 160 per tile
    PKW = 2 * NK               # 320 per pair
    NSL = NT * NS              # per-pixel-slot count (128)

    _force_sigma_tables()
    nc = bacc.Bacc("TRN2", target_bir_lowering=False, debug=False)

    XDT = mybir.dt.float8e4 if X_FP8 else BF16
    xT_d = nc.dram_tensor("xT", [NT * D, TP], XDT, kind="ExternalInput").ap()
    w_d = nc.dram_tensor("w", [128, 2 * M], BF16, kind="ExternalInput").ap()
    vp_d = nc.dram_tensor("vp", [128, NT * C * NS], F16,
                          kind="ExternalInput").ap()
    q_d = nc.dram_tensor("q", [128, NT * 2 * NK], F16,
                         kind="ExternalInput").ap()
    msk_d = nc.dram_tensor("msk", [128, NT * 2 * NKC], F16,
                           kind="ExternalInput").ap()
    if with_bias:
        bias_d = nc.dram_tensor("bias", [1, M], BF16, kind="ExternalInput").ap()
    szf_d = nc.dram_tensor("szf", [128, 2 * NSL], F32,
                           kind="ExternalOutput").ap()

    with tile.TileContext(nc) as tc, \
            tc.tile_pool(name="const", bufs=1) as cpool, \
            tc.tile_pool(name="xin", bufs=4) as xpool, \
            tc.tile_pool(name="ps", bufs=2, space="PSUM") as pspool, \
            tc.tile_pool(name="p2", bufs=2) as pp, \
            tc.tile_pool(name="p4", bufs=4) as pp4, \
            tc.tile_pool(name="p3", bufs=3) as pp3:

        w_sb = cpool.tile([128, 2 * M], BF16)
        vp_sb = cpool.tile([128, NT * C * NS], F16)
        q_sb = cpool.tile([128, NT * 2 * NK], F16)
        msk_sb = cpool.tile([128, NT * 2 * NKC], F16)
        szf = cpool.tile([128, 2 * NSL], F32)
        gate = cpool.tile([128, 1], F32)
        if with_bias:
            bias_sb = cpool.tile([1, M], BF16)
            nc.sync.dma_start(out=bias_sb, in_=bias_d)
            ones_sb = cpool.tile([1, 128], BF16)
            nc.vector.memset(ones_sb, 1.0)

        st = [dict() for _ in range(NT)]
        sp = [dict() for _ in range(NP)]

        def dma_tile(j):
            # both D-halves in one DMA: [xt0 | xt1] side by side
            xt = xpool.tile([128, 2 * TP], XDT, tag="xt")
            nc.sync.dma_start(out=xt, in_=bass.AP(
                tensor=xT_d.tensor, offset=j * D * TP,
                ap=[[TP, 128], [128 * TP, 2], [1, TP]]))
            st[j]["xt"] = xt

        def s_mm(j):
            xt = st[j].pop("xt")
            xt0, xt1 = xt[:, 0:TP], xt[:, TP:2 * TP]
            ps = pspool.tile([128, 2048], F32, tag="ps")
            for sub in range(NS):
                o = ps[:, sub * 128:sub * 128 + M]
                if with_bias:
                    nc.tensor.matmul(o, ones_sb[:, :], bias_sb[:, :],
                                     start=True, stop=False)
                nc.tensor.matmul(o, xt0[:, sub * 128:(sub + 1) * 128],
                                 w_sb[:, 0:M], start=not with_bias, stop=False)
                nc.tensor.matmul(o, xt1[:, sub * 128:(sub + 1) * 128],
                                 w_sb[:, M:2 * M], start=False, stop=True)
            st[j]["ps"] = ps

        # PSUM field views.  W columns (host-permuted, field-major):
        #   [loc(c,k) 0:30 | s_raw(c,k) 30:60 | cc(c,k) 60:90 | logit 90:100]
        # All epilogue tensors keep this natural (s, c, k) order, k innermost.
        def ps_sck(ps, off):
            return _v(ps[:, :], off, [[128, NS], [K, C], [1, K]])

        def ps_sk(ps, off):
            return _v(ps[:, :], off, [[128, NS], [1, K]])

        def t_sck(t8, tl):         # (s,c,k) tile-slice of a pair tensor
            return _v(t8[:, :], tl * NKC, [[C * K, NS], [K, C], [1, K]])

        def t_sk(t8, tl):          # (s,k) tile-slice
            return _v(t8[:, :], tl * NK, [[K, NS], [1, K]])

        def cs(t8, c):             # (t,s,k) view of c-slice of pair (t,s,c,k)
            return _v(t8[:, :], c * K, [[NKC, 2], [C * K, NS], [1, K]])

        def ksl(t8, off, n):       # (t,s,k) reduce view, k innermost
            return _v(t8[:, :], off, [[NK, n], [K, NS], [1, K]])

        def s_ext(j):
            p, tl = j // 2, j % 2
            last = (p == NP - 1)
            if tl == 0:
                sp[p]["ccf"] = pp.tile([128, 2 * 640], F16, tag="ccf",
                                       name="ccf")
                sp[p]["A"] = pp.tile([128, PW], F16, tag="A", name="A")
                if not last:
                    sp[p]["es"] = pp.tile([128, PW], F32, tag="es", name="es")
            ps = st[j].pop("ps")
            ccf, A = sp[p]["ccf"], sp[p]["A"]
            # ACT: es = exp(s_raw) out of PSUM; contiguous writes throughout.
            # The LAST pair gets per-tile es so its scale chain can start on
            # tile NT-2 while tile NT-1 is still in the matmul (shorter tail).
            if last:
                esh = pp.tile([128, NKC], F32, tag="esh", name="esh")
                nc.scalar.activation(
                    _v(esh[:, :], 0, [[C * K, NS], [K, C], [1, K]]),
                    ps_sck(ps, 30), AF.Exp)
                sp[p]["es%d" % tl] = esh
            else:
                nc.scalar.activation(t_sck(sp[p]["es"], tl), ps_sck(ps, 30),
                                     AF.Exp)
            # coeff + logit columns evacuated in ONE contiguous 640-wide copy
            nc.scalar.activation(
                _v(ccf[:, :], tl * 640, [[40, NS], [1, 40]]),
                _v(ps[:, :], 60, [[128, NS], [1, 40]]), AF.Copy)
            # DVE: A = loc' + vp2 (PSUM f32 read, 1x)
            vpv = _v(vp_sb[:, :], j * C * NS, [[1, NS], [NS, C], [0, K]])
            nc.vector.tensor_tensor(t_sck(A, tl), ps_sck(ps, 0), vpv, AL.add)
            if tl == 1:
                # ACT: ef = exp(logits); DVE: sf = sum_k e^{f0} to output
                f0v = _v(ccf[:, :], 30, [[640, 2], [40, NS], [1, K]])
                ef = pp4.tile([128, PKW], F16, tag="ef")
                nc.scalar.activation(
                    _v(ef[:, :], 0, [[NK, 2], [K, NS], [1, K]]), f0v, AF.Exp)
                sp[p]["ef"] = ef
                nc.vector.tensor_reduce(
                    szf[:, NSL + p * 2 * NS:NSL + (p + 1) * 2 * NS],
                    ksl(ef, 0, 2), axis=AX.X, op=AL.add)

        def s_chain_half(p, tl):
            # per-tile ls/lr/rt for the last pair; rt halves land in one
            # pair-wide rt tile
            esh = sp[p].pop("es%d" % tl)
            if tl == 0:
                sp[p]["rt"] = pp.tile([128, PW], F16, tag="rt", name="rt")
            rt = sp[p]["rt"]
            lsh = pp.tile([128, NKC], F32, tag="lsh")
            nc.scalar.activation(lsh, esh, AF.Ln, bias=1.0)
            lrh = pp.tile([128, NKC], F32, tag="esh")   # reuse esh slots
            nc.scalar.activation(lrh, lsh, AF.Ln, scale=127.5)
            nc.scalar.activation(rt[:, tl * NKC:(tl + 1) * NKC], lrh,
                                 AF.Exp, scale=-1.0)
            if p == NP - 1 and tl == 1:
                # zero gate written by the last nle-phase ACT op: pins all
                # sigmas after the natural_log_exp work (one table switch)
                nc.scalar.mul(gate, rt[:, 0:1], 0.0)

        def s_pair_a(p):
            ccf, A = sp[p]["ccf"], sp[p]["A"]
            def cs_cc(c):          # (t,s,k) view of coeff c inside ccf
                return _v(ccf[:, :], c * K, [[640, 2], [40, NS], [1, K]])
            if p < NP - 1:
                es = sp[p]["es"]
                # ACT scale chain: ls = ln(1+es); lr = ln(127.5*ls);
                # rt = exp(-lr) = 1/(127.5*softplus(s_raw))
                ls = pp.tile([128, PW], F32, tag="ls")
                nc.scalar.activation(ls, es, AF.Ln, bias=1.0)
                lr = pp.tile([128, PW], F32, tag="es")   # reuse es slot pair
                nc.scalar.activation(lr, ls, AF.Ln, scale=127.5)
                rt = pp.tile([128, PW], F16, tag="rt")
                nc.scalar.activation(rt, lr, AF.Exp, scale=-1.0)
                sp[p]["rt"] = rt
            else:
                s_chain_half(p, 1)
            # DVE: coupling  A1 += q0*cc0, A2 += q0*cc1 + q1*cc2
            # (q host-materialized over k so every op keeps f16 2x mode)
            h = pp.tile([128, 3 * PKW], F16, tag="h")
            qv = [_v(q_sb[:, :], p * 2 * (2 * NK) + jd * NK,
                     [[2 * NK, 2], [K, NS], [1, K]]) for jd in (0, 1)]
            hv = [_v(h[:, :], i * PKW, [[NK, 2], [K, NS], [1, K]])
                  for i in range(3)]
            nc.vector.tensor_tensor(hv[0], qv[0], cs_cc(0), AL.mult)
            nc.vector.tensor_tensor(hv[1], qv[0], cs_cc(1), AL.mult)
            nc.vector.tensor_tensor(hv[2], qv[1], cs_cc(2), AL.mult)
            nc.vector.tensor_tensor(cs(A, 1), cs(A, 1), hv[0], AL.add)
            nc.vector.tensor_tensor(cs(A, 2), cs(A, 2), hv[1], AL.add)
            nc.vector.tensor_tensor(cs(A, 2), cs(A, 2), hv[2], AL.add)

        def s_pair_b(p):
            A, rt = sp[p].pop("A"), sp[p].pop("rt")
            sp[p].pop("ccf")  # dead
            p0 = pp.tile([128, PW], F16, tag="p0")
            nc.vector.tensor_tensor(p0, A, rt, AL.mult)
            m0 = pp.tile([128, PW], F16, tag="A")     # reuse A slot pair
            nc.vector.tensor_tensor(m0, p0, rt, AL.subtract)
            X = pp4.tile([128, 2 * PW], F16, tag="X")
            pkv = _v(msk_sb[:, :], p * 2 * (2 * NKC),
                     [[2 * NKC, 2], [1, NKC]])
            mkv = _v(msk_sb[:, :], p * 2 * (2 * NKC) + NKC,
                     [[2 * NKC, 2], [1, NKC]])
            nc.vector.tensor_tensor(
                _v(X[:, :], 0, [[NKC, 2], [1, NKC]]),
                _v(p0[:, :], 0, [[NKC, 2], [1, NKC]]), pkv, AL.add)
            nc.vector.tensor_tensor(
                _v(X[:, :], PW, [[NKC, 2], [1, NKC]]),
                _v(m0[:, :], 0, [[NKC, 2], [1, NKC]]), mkv, AL.subtract)
            sp[p]["X"] = X

        def s_pair_c(p):
            X = sp[p].pop("X")
            ef = sp[p].pop("ef")
            sg = pp3.tile([128, 2 * PW], F32, tag="sg")
            # gate (zero bias written by the LAST nle-phase ACT op) pins every
            # sigma after all natural_log_exp work: exactly one table switch
            nc.scalar.activation(sg, X, AF.Sigmoid, bias=gate[:, 0:1])
            d = pp.tile([128, PW], F16, tag="d")
            nc.vector.tensor_tensor(d, sg[:, 0:PW], sg[:, PW:2 * PW],
                                    AL.subtract)
            # mixture weights as pure products: ez = e^{f0} * d0*d1*d2
            p1 = pp.tile([128, PKW], F16, tag="p1")
            nc.vector.tensor_tensor(p1[:, :].rearrange("p (t s k) -> p t s k",
                                                       t=2, s=NS),
                                    cs(d, 0), cs(d, 1), AL.mult)
            p2 = pp.tile([128, PKW], F32, tag="p2")
            nc.vector.tensor_tensor(p2[:, :].rearrange("p (t s k) -> p t s k",
                                                       t=2, s=NS),
                                    _v(p1[:, :], 0, [[NK, 2], [K, NS], [1, K]]),
                                    cs(d, 2), AL.mult)
            ez = pp.tile([128, PKW], F32, tag="ez")
            nc.vector.tensor_tensor(ez, p2, ef, AL.mult)
            nc.vector.tensor_reduce(szf[:, p * 2 * NS:(p + 1) * 2 * NS],
                                    ksl(ez, 0, 2), axis=AX.X, op=AL.add)

        # ---- emission ----
        # xt0 + w gate the first matmul: xt0 first on Sync, w via the Pool
        # sequencer (fast DGE dispatch).  The big epilogue inputs (q, msk)
        # are deferred behind the first xt tiles so they don't steal DMA
        # bandwidth from the matmul stream they'd otherwise stall.
        dma_tile(0)
        nc.gpsimd.dma_start(out=w_sb, in_=w_d)
        dma_tile(1)
        nc.gpsimd.dma_start(out=vp_sb, in_=vp_d)
        dma_tile(2)
        for j in range(NT):
            if j + 3 < NT:
                dma_tile(j + 3)
            if j % 2 == 0:
                # stream mask/q inputs per-pair, just-in-time, so they never
                # stall the matmul xt stream
                p = j // 2
                nc.gpsimd.dma_start(
                    out=msk_sb[:, p * 2 * PW:(p + 1) * 2 * PW],
                    in_=msk_d[:, p * 2 * PW:(p + 1) * 2 * PW])
                nc.gpsimd.dma_start(
                    out=q_sb[:, p * 2 * PKW:(p + 1) * 2 * PKW],
                    in_=q_d[:, p * 2 * PKW:(p + 1) * 2 * PKW])
            s_mm(j)
            s_ext(j)
            if j == NT - 2:
                s_chain_half(NP - 1, 0)
            if j % 2 == 1:
                s_pair_a(j // 2)
                s_pair_b(j // 2)
        for p in range(NP):
            s_pair_c(p)         # ACT: all sigmas grouped -> one table switch
        nc.sync.dma_start(out=szf_d, in_=szf)

    nc.compile()
    return nc


_CACHE = {}


def _get_program(pix, with_bias):
    key = (pix, with_bias)
    if key not in _CACHE:
        _CACHE[key] = build_program(pix, with_bias)
    return _CACHE[key]


def _permute_cols(Wr):
    """[.., K, 10] field-blocks -> field-major columns
    [loc(c,k) | s(c,k) | cc(c,k) | logit(k)], with prescales folded:
    loc *= -127.5 (A = vp2 - 127.5*locs_m), cc *= -1 (h = q*cc')."""
    parts = []
    for c in range(C):
        parts.append(Wr[..., :, 1 + c] * -127.5)     # loc_c over k
    for c in range(C):
        parts.append(Wr[..., :, 4 + c])              # s_c over k
    for c in range(C):
        parts.append(Wr[..., :, 7 + c] * -1.0)       # cc_c over k
    parts.append(Wr[..., :, 0])                      # logit over k
    return np.concatenate(parts, axis=-1)


def _prescale_w(W_conv):
    Wr = W_conv.astype(np.float64).reshape(D, K, 10)
    Wp = _permute_cols(Wr)                           # [D, 100]
    wsb = Wp.reshape(2, 128, M).transpose(1, 0, 2).reshape(128, 2 * M)
    return np.ascontiguousarray(wsb.astype(ml_dtypes.bfloat16))


def _pack_host(vf, per):
    """vf [per,3] raw 0..255 -> (vp, q, msk) host tensors for one core.

    Pixel local index = t*2048 + s*128 + partition; partition-major:
      vp  [128, NT*C*NS]   vp2 = v-127                  at t*48 + c*16 + s
      q   [128, NT*2*160]  (vp2_j - 0.5) k-materialized at t*320 + j*160
                                                          + s*10 + k
      msk [128, NT*2*480]  PK=32768*(v==255) k-mat    at t*960 + s*30+c*10+k
                           MK=32768*(v==0)   k-mat    at t*960+480 + ...
    """
    NT = per // 2048
    v = vf.reshape(NT, NS, 128, C).transpose(2, 0, 3, 1)   # [128, t, c, s]
    vp = np.ascontiguousarray(
        (v - 127.0).reshape(128, -1).astype(np.float16))
    vs = v.transpose(0, 1, 3, 2)                           # [128, t, s, c]
    qk = np.broadcast_to((vs[:, :, None, :, 0:2] - 127.5).transpose(
        0, 1, 2, 4, 3)[..., None], (128, NT, 1, 2, NS, K)) # [128,t,1,j,s,k]
    q = np.ascontiguousarray(qk.reshape(128, -1).astype(np.float16))
    pk = KBIG * (vs == 255.0)                              # [128, t, s, c]
    mk = KBIG * (vs == 0.0)
    msk = np.stack([pk, mk], axis=2)                       # [128, t, b, s, c]
    msk = np.broadcast_to(msk[..., None], (128, NT, 2, NS, C, K))
    msk = np.ascontiguousarray(msk.reshape(128, -1).astype(np.float16))
    return vp, q, msk


def shard_inputs(x, value, W_conv, b_conv, n_cores=N_CORES):
    B = x.shape[0]
    pix_total = B * x.shape[1] * x.shape[2]
    per = pix_total // n_cores
    xf = np.ascontiguousarray(x.reshape(pix_total, D).astype(np.float32))
    vf = value.reshape(pix_total, C).astype(np.float32)
    w_bf = _prescale_w(np.asarray(W_conv))
    with_bias = bool(np.any(b_conv))
    in_maps = []
    for i in range(n_cores):
        xdt = ml_dtypes.float8_e4m3fn if X_FP8 else ml_dtypes.bfloat16
        xT = xf[i * per:(i + 1) * per].T.astype(xdt)
        # tile-contiguous layout [NT*D, 2048]: each tile DMA one dense block
        xT = np.ascontiguousarray(
            xT.reshape(D, per // 2048, 2048).transpose(1, 0, 2)
        ).reshape(-1, 2048)
        vp, q, msk = _pack_host(vf[i * per:(i + 1) * per], per)
        mm = {"xT": xT, "w": w_bf, "vp": vp, "q": q, "msk": msk}
        if with_bias:
            br = _permute_cols(
                b_conv.astype(np.float64).reshape(K, 10)[None, :, :])
            mm["bias"] = br.reshape(1, M).astype(ml_dtypes.bfloat16)
        in_maps.append(mm)
    return in_maps, with_bias, per


def _finish_host(szf, per):
    """szf [128, 2*NSL] f32 -> per-image mix_lp sums [NIMG] (f64 math)."""
    NSL = per // 128
    sz = szf[:, 0:NSL].astype(np.float64)
    sf = szf[:, NSL:2 * NSL].astype(np.float64)
    mix = np.log(np.maximum(sz, 1e-300)) - np.log(sf)
    # pixel = t*2048 + s*128 + p ; image = pixel // 1024 -> (t, s//8)
    NT = per // 2048
    return mix.reshape(128, NT, 2, 8).sum(axis=(0, 3)).reshape(-1)


def kernel(x, value, W_conv, b_conv):
    x = np.asarray(x)
    value = np.asarray(value)
    W_conv = np.asarray(W_conv)
    b_conv = np.asarray(b_conv)
    in_maps, with_bias, per = shard_inputs(x, value, W_conv, b_conv)
    nc = _get_program(per, with_bias)
    res = run_bass_kernel_spmd(nc, in_maps, list(range(N_CORES)))
    parts = [
        _finish_host(res.results[i]["szf"], per).astype(np.float32)
        for i in range(N_CORES)
    ]
    return np.concatenate(parts)


# revision 28
# speedup vs baseline: 1.1604x; 1.1604x over previous
"""Trainium2 Bass kernel for nn_LogisticMixture (discretized logistic mixture loss).

Contract: kernel(**inputs) takes FULL unsharded numpy inputs
  x      [128, 32, 32, 256] f32
  value  [128, 32, 32, 3]   f32 (integer pixel values 0..255)
  W_conv [256, 100]         f32
  b_conv [100]              f32
and returns the full [128] f32 output (per-image sum of mixture log-probs).

Strategy: pure data parallelism over batch across 8 NeuronCores (16384
pixels/core).  Matmul identical to the proven baseline (x^T-stationary,
W streamed, 2048-px macro-tiles of 16x128-px subtiles).  The epilogue is
built on a sigmoid-difference identity with a products-not-logs mixture:

  d_{k,c} = sigma(X1) - sigma(X2)        (= discretized logistic prob)
  X1 = p + PK,  X2 = (p - r_t) - MK,     p = A * r_t
  A   = vp2 - 127.5*locs_m  (host-prescaled W columns, vp2 = v-127)
  r_t = 1/(127.5*softplus(s_raw))  via ACT chain exp->ln1p->ln->exp
  PK  = 32768*(v==255), MK = 32768*(v==0): sigma saturates EXACTLY to
  1/0 at +-32768, so boundary pixels reduce to cdf / survival with no
  masking ops.
  sz  = sum_k e^{f0_k} * d_k0*d_k1*d_k2 ,  sf = sum_k e^{f0_k}
  mix_lp = ln(sz) - ln(sf)   <- the ONLY logs, done on the HOST from the
  tiny [128, px/128] sz/sf outputs; the per-(k,c) ln and the mixture exp
  cancel, killing the whole log-sum-exp chain on-chip.

Layouts follow the PSUM's natural (s, c, k) order (s = px-subtile slot,
k innermost) so every PSUM evacuation is a contiguous-write ACT op
(strided writes measured 5x slower).  Masks and coupling q are
k-materialized host-side so all fat DVE ops keep the 2x f16 mode
(0.56 ns/elem measured).  ACT needs exactly two tables: natural_log_exp
for the scale chain, sigmoid for the single sigma op per pair; all
sigmas run after the last extraction so there is ~1 switch.
Engines: PE matmul; ACT activations + PSUM copies; DVE f16 elementwise
+ f32 products; Pool a slice of the f32 sigma-subtract.
"""
import sys
import os

for _p in ("/opt/trn_rl_repo", "/root/.axon_site/_ro/trn_rl_repo"):
    if os.path.isdir(_p) and _p not in sys.path:
        sys.path.append(_p)

import numpy as np
import ml_dtypes

import concourse.bass as bass
import concourse.mybir as mybir
import concourse.tile as tile
from concourse import bacc
from concourse.bass_utils import run_bass_kernel_spmd
import concourse.hw_specs as hw_specs

F32 = mybir.dt.float32
F16 = mybir.dt.float16
BF16 = mybir.dt.bfloat16
AL = mybir.AluOpType
AF = mybir.ActivationFunctionType
AX = mybir.AxisListType

N_CORES = 8
X_FP8 = True     # xT in float8e4 (e4m3): halves the dominant DMA stream
D = 256
M = 100          # NUM_MIX * NUM_OUT
K = 10           # mixtures
C = 3            # channels
NS = 16          # 128-px subtiles per macro-tile
KBIG = 32768.0   # mask shift; sigma(+-32768) saturates exactly


def _force_sigma_tables():
    """Keep only sigmoid_and_others + natural_log_exp_and_others non-empty
    so the act-table chooser never picks a set lacking our functions; dict
    order is preserved so set ids stay aligned with act_info.json."""
    if getattr(hw_specs, "_ant_sigma_tables", False):
        return
    orig = hw_specs.get_activation_tables
    import functools

    @functools.cache
    def patched(arch):
        tabs = dict(orig(arch))
        keep = {"sigmoid_and_others", "natural_log_exp_and_others"}
        return {k: (set(v) if k in keep else set()) for k, v in tabs.items()}

    hw_specs.get_activation_tables = patched
    bacc.get_activation_tables = patched
    hw_specs._ant_sigma_tables = True


def _v(ap0, offset, pattern):
    """AP on a tile's [:, :] AP: keep partition dim, replace free dims
    (stride-0 broadcast dims allowed on non-innermost positions)."""
    if not isinstance(ap0, bass.AP):
        ap0 = ap0[:, :]
    return bass.AP(tensor=ap0.tensor, offset=ap0.offset + offset,
                   ap=[list(ap0.ap[0])] + [list(p) for p in pattern])


def build_program(pix=16384, with_bias=False):
    """Single-core SPMD program. pix must be a multiple of 4096."""
    TP = 2048                  # pixels per macro-tile (2 images)
    NT = pix // TP             # macro-tiles
    NP = NT // 2               # 2-tile pairs for the fat epilogue ops
    NKC = NS * K * C           # 480 per tile
    PW = 2 * NKC               # 960 per pair
    NK = NS * K                # 160 per tile
    PKW = 2 * NK               # 320 per pair
    NSL = NT * NS              # per-pixel-slot count (128)

    _force_sigma_tables()
    nc = bacc.Bacc("TRN2", target_bir_lowering=False, debug=False)

    XDT = mybir.dt.float8e4 if X_FP8 else BF16
    xT_d = nc.dram_tensor("xT", [NT * D, TP], XDT, kind="ExternalInput").ap()
    w_d = nc.dram_tensor("w", [128, 2 * M], BF16, kind="ExternalInput").ap()
    vp_d = nc.dram_tensor("vp", [128, NT * C * NS], F16,
                          kind="ExternalInput").ap()
    q_d = nc.dram_tensor("q", [128, NT * 2 * NK], F16,
                         kind="ExternalInput").ap()
    msk_d = nc.dram_tensor("msk", [128, NT * 2 * NKC], F16,
                           kind="ExternalInput").ap()
    if with_bias:
        bias_d = nc.dram_tensor("bias", [1, M], BF16, kind="ExternalInput").ap()
    szf_d = nc.dram_tensor("szf", [128, 2 * NSL], F32,
                           kind="ExternalOutput").ap()

    with tile.TileContext(nc) as tc, \
            tc.tile_pool(name="const", bufs=1) as cpool, \
            tc.tile_pool(name="xin", bufs=4) as xpool, \
            tc.tile_pool(name="ps", bufs=2, space="PSUM") as pspool, \
            tc.tile_pool(name="p2", bufs=2) as pp, \
            tc.tile_pool(name="p4", bufs=4) as pp4, \
            tc.tile_pool(name="p3", bufs=3) as pp3:

        w_sb = cpool.tile([128, 2 * M], BF16)
        vp_sb = cpool.tile([128, NT * C * NS], F16)
        q_sb = cpool.tile([128, NT * 2 * NK], F16)
        msk_sb = cpool.tile([128, NT * 2 * NKC], F16)
        szf = cpool.tile([128, 2 * NSL], F32)
        gate = cpool.tile([128, 1], F32)
        if with_bias:
            bias_sb = cpool.tile([1, M], BF16)
            nc.sync.dma_start(out=bias_sb, in_=bias_d)
            ones_sb = cpool.tile([1, 128], BF16)
            nc.vector.memset(ones_sb, 1.0)

        st = [dict() for _ in range(NT)]
        sp = [dict() for _ in range(NP)]

        def dma_tile(j):
            # both D-halves in one DMA: [xt0 | xt1] side by side
            xt = xpool.tile([128, 2 * TP], XDT, tag="xt")
            nc.sync.dma_start(out=xt, in_=bass.AP(
                tensor=xT_d.tensor, offset=j * D * TP,
                ap=[[TP, 128], [128 * TP, 2], [1, TP]]))
            st[j]["xt"] = xt

        def s_mm(j):
            xt = st[j].pop("xt")
            xt0, xt1 = xt[:, 0:TP], xt[:, TP:2 * TP]
            ps = pspool.tile([128, 2048], F32, tag="ps")
            for sub in range(NS):
                o = ps[:, sub * 128:sub * 128 + M]
                if with_bias:
                    nc.tensor.matmul(o, ones_sb[:, :], bias_sb[:, :],
                                     start=True, stop=False)
                nc.tensor.matmul(o, xt0[:, sub * 128:(sub + 1) * 128],
                                 w_sb[:, 0:M], start=not with_bias, stop=False)
                nc.tensor.matmul(o, xt1[:, sub * 128:(sub + 1) * 128],
                                 w_sb[:, M:2 * M], start=False, stop=True)
            st[j]["ps"] = ps

        # PSUM field views.  W columns (host-permuted, field-major):
        #   [loc(c,k) 0:30 | s_raw(c,k) 30:60 | cc(c,k) 60:90 | logit 90:100]
        # All epilogue tensors keep this natural (s, c, k) order, k innermost.
        def ps_sck(ps, off):
            return _v(ps[:, :], off, [[128, NS], [K, C], [1, K]])

        def ps_sk(ps, off):
            return _v(ps[:, :], off, [[128, NS], [1, K]])

        def t_sck(t8, tl):         # (s,c,k) tile-slice of a pair tensor
            return _v(t8[:, :], tl * NKC, [[C * K, NS], [K, C], [1, K]])

        def t_sk(t8, tl):          # (s,k) tile-slice
            return _v(t8[:, :], tl * NK, [[K, NS], [1, K]])

        def cs(t8, c):             # (t,s,k) view of c-slice of pair (t,s,c,k)
            return _v(t8[:, :], c * K, [[NKC, 2], [C * K, NS], [1, K]])

        def ksl(t8, off, n):       # (t,s,k) reduce view, k innermost
            return _v(t8[:, :], off, [[NK, n], [K, NS], [1, K]])

        def s_ext(j):
            p, tl = j // 2, j % 2
            last = (p == NP - 1)
            if tl == 0:
                sp[p]["ccf"] = pp.tile([128, 2 * 640], F16, tag="ccf",
                                       name="ccf")
                sp[p]["A"] = pp.tile([128, PW], F16, tag="A", name="A")
                if not last:
                    sp[p]["es"] = pp.tile([128, PW], F32, tag="es", name="es")
            ps = st[j].pop("ps")
            ccf, A = sp[p]["ccf"], sp[p]["A"]
            # ACT: es = exp(s_raw) out of PSUM; contiguous writes throughout.
            # The LAST pair gets per-tile es so its scale chain can start on
            # tile NT-2 while tile NT-1 is still in the matmul (shorter tail).
            if last:
                esh = pp.tile([128, NKC], F32, tag="esh", name="esh")
                nc.scalar.activation(
                    _v(esh[:, :], 0, [[C * K, NS], [K, C], [1, K]]),
                    ps_sck(ps, 30), AF.Exp)
                sp[p]["es%d" % tl] = esh
            else:
                nc.scalar.activation(t_sck(sp[p]["es"], tl), ps_sck(ps, 30),
                                     AF.Exp)
            # coeff + logit columns evacuated in ONE contiguous 640-wide copy
            nc.scalar.activation(
                _v(ccf[:, :], tl * 640, [[40, NS], [1, 40]]),
                _v(ps[:, :], 60, [[128, NS], [1, 40]]), AF.Copy)
            # DVE: A = loc' + vp2 (PSUM f32 read, 1x)
            vpv = _v(vp_sb[:, :], j * C * NS, [[1, NS], [NS, C], [0, K]])
            nc.vector.tensor_tensor(t_sck(A, tl), ps_sck(ps, 0), vpv, AL.add)
            if tl == 1:
                # ACT: ef = exp(logits); DVE: sf = sum_k e^{f0} to output
                f0v = _v(ccf[:, :], 30, [[640, 2], [40, NS], [1, K]])
                ef = pp4.tile([128, PKW], F16, tag="ef")
                nc.scalar.activation(
                    _v(ef[:, :], 0, [[NK, 2], [K, NS], [1, K]]), f0v, AF.Exp)
                sp[p]["ef"] = ef
                nc.vector.tensor_reduce(
                    szf[:, NSL + p * 2 * NS:NSL + (p + 1) * 2 * NS],
                    ksl(ef, 0, 2), axis=AX.X, op=AL.add)

        def s_chain_half(p, tl):
            # per-tile ls/lr/rt for the last pair; rt halves land in one
            # pair-wide rt tile
            esh = sp[p].pop("es%d" % tl)
            if tl == 0:
                sp[p]["rt"] = pp.tile([128, PW], F16, tag="rt", name="rt")
            rt = sp[p]["rt"]
            lsh = pp.tile([128, NKC], F32, tag="lsh")
            nc.scalar.activation(lsh, esh, AF.Ln, bias=1.0)
            lrh = pp.tile([128, NKC], F32, tag="esh")   # reuse esh slots
            nc.scalar.activation(lrh, lsh, AF.Ln, scale=127.5)
            nc.scalar.activation(rt[:, tl * NKC:(tl + 1) * NKC], lrh,
                                 AF.Exp, scale=-1.0)
            if p == NP - 1 and tl == 1:
                # zero gate written by the last nle-phase ACT op: pins all
                # sigmas after the natural_log_exp work (one table switch)
                nc.scalar.mul(gate, rt[:, 0:1], 0.0)

        def s_pair_a(p):
            ccf, A = sp[p]["ccf"], sp[p]["A"]
            def cs_cc(c):          # (t,s,k) view of coeff c inside ccf
                return _v(ccf[:, :], c * K, [[640, 2], [40, NS], [1, K]])
            if p < NP - 1:
                es = sp[p]["es"]
                # ACT scale chain: ls = ln(1+es); lr = ln(127.5*ls);
                # rt = exp(-lr) = 1/(127.5*softplus(s_raw))
                ls = pp.tile([128, PW], F32, tag="ls")
                nc.scalar.activation(ls, es, AF.Ln, bias=1.0)
                lr = pp.tile([128, PW], F32, tag="es")   # reuse es slot pair
                nc.scalar.activation(lr, ls, AF.Ln, scale=127.5)
                rt = pp.tile([128, PW], F16, tag="rt")
                nc.scalar.activation(rt, lr, AF.Exp, scale=-1.0)
                sp[p]["rt"] = rt
            else:
                s_chain_half(p, 1)
            # DVE: coupling  A1 += q0*cc0, A2 += q0*cc1 + q1*cc2
            # (q host-materialized over k so every op keeps f16 2x mode)
            h = pp.tile([128, 3 * PKW], F16, tag="h")
            qv = [_v(q_sb[:, :], p * 2 * (2 * NK) + jd * NK,
                     [[2 * NK, 2], [K, NS], [1, K]]) for jd in (0, 1)]
            hv = [_v(h[:, :], i * PKW, [[NK, 2], [K, NS], [1, K]])
                  for i in range(3)]
            nc.vector.tensor_tensor(hv[0], qv[0], cs_cc(0), AL.mult)
            nc.vector.tensor_tensor(hv[1], qv[0], cs_cc(1), AL.mult)
            nc.vector.tensor_tensor(hv[2], qv[1], cs_cc(2), AL.mult)
            nc.vector.tensor_tensor(cs(A, 1), cs(A, 1), hv[0], AL.add)
            nc.vector.tensor_tensor(cs(A, 2), cs(A, 2), hv[1], AL.add)
            nc.vector.tensor_tensor(cs(A, 2), cs(A, 2), hv[2], AL.add)

        def s_pair_b(p):
            A, rt = sp[p].pop("A"), sp[p].pop("rt")
            sp[p].pop("ccf")  # dead
            p0 = pp.tile([128, PW], F16, tag="p0")
            nc.vector.tensor_tensor(p0, A, rt, AL.mult)
            m0 = pp.tile([128, PW], F16, tag="A")     # reuse A slot pair
            nc.vector.tensor_tensor(m0, p0, rt, AL.subtract)
            X = pp4.tile([128, 2 * PW], F16, tag="X")
            pkv = _v(msk_sb[:, :], p * 2 * (2 * NKC),
                     [[2 * NKC, 2], [1, NKC]])
            mkv = _v(msk_sb[:, :], p * 2 * (2 * NKC) + NKC,
                     [[2 * NKC, 2], [1, NKC]])
            nc.vector.tensor_tensor(
                _v(X[:, :], 0, [[NKC, 2], [1, NKC]]),
                _v(p0[:, :], 0, [[NKC, 2], [1, NKC]]), pkv, AL.add)
            nc.vector.tensor_tensor(
                _v(X[:, :], PW, [[NKC, 2], [1, NKC]]),
                _v(m0[:, :], 0, [[NKC, 2], [1, NKC]]), mkv, AL.subtract)
            sp[p]["X"] = X

        def s_pair_c(p):
            X = sp[p].pop("X")
            ef = sp[p].pop("ef")
            sg = pp3.tile([128, 2 * PW], F32, tag="sg")
            # gate (zero bias written by the LAST nle-phase ACT op) pins every
            # sigma after all natural_log_exp work: exactly one table switch
            nc.scalar.activation(sg, X, AF.Sigmoid, bias=gate[:, 0:1])
            d = pp.tile([128, PW], F16, tag="d")
            nc.vector.tensor_tensor(d, sg[:, 0:PW], sg[:, PW:2 * PW],
                                    AL.subtract)
            # mixture weights as pure products: ez = e^{f0} * d0*d1*d2
            p1 = pp.tile([128, PKW], F16, tag="p1")
            nc.vector.tensor_tensor(p1[:, :].rearrange("p (t s k) -> p t s k",
                                                       t=2, s=NS),
                                    cs(d, 0), cs(d, 1), AL.mult)
            p2 = pp.tile([128, PKW], F32, tag="p2")
            nc.vector.tensor_tensor(p2[:, :].rearrange("p (t s k) -> p t s k",
                                                       t=2, s=NS),
                                    _v(p1[:, :], 0, [[NK, 2], [K, NS], [1, K]]),
                                    cs(d, 2), AL.mult)
            ez = pp.tile([128, PKW], F32, tag="ez")
            nc.vector.tensor_tensor(ez, p2, ef, AL.mult)
            nc.vector.tensor_reduce(szf[:, p * 2 * NS:(p + 1) * 2 * NS],
                                    ksl(ez, 0, 2), axis=AX.X, op=AL.add)

        # ---- emission ----
        # xt0 + w gate the first matmul: xt0 first on Sync, w via the Pool
        # sequencer (fast DGE dispatch).  The big epilogue inputs (q, msk)
        # are deferred behind the first xt tiles so they don't steal DMA
        # bandwidth from the matmul stream they'd otherwise stall.
        dma_tile(0)
        nc.gpsimd.dma_start(out=w_sb, in_=w_d)
        dma_tile(1)
        nc.gpsimd.dma_start(out=vp_sb, in_=vp_d)
        dma_tile(2)
        for j in range(NT):
            if j + 3 < NT:
                dma_tile(j + 3)
            if j % 2 == 0:
                # stream mask/q inputs per-pair, just-in-time, so they never
                # stall the matmul xt stream
                p = j // 2
                nc.gpsimd.dma_start(
                    out=msk_sb[:, p * 2 * PW:(p + 1) * 2 * PW],
                    in_=msk_d[:, p * 2 * PW:(p + 1) * 2 * PW])
                nc.gpsimd.dma_start(
                    out=q_sb[:, p * 2 * PKW:(p + 1) * 2 * PKW],
                    in_=q_d[:, p * 2 * PKW:(p + 1) * 2 * PKW])
            s_mm(j)
            s_ext(j)
            if j == NT - 2:
                s_chain_half(NP - 1, 0)
            if j % 2 == 1:
                s_pair_a(j // 2)
            # pair_b emitted two tiles late: extraction (which releases
            # PSUM for the next matmul) outranks it in the DVE queue
            if j % 2 == 1 and j >= 3:
                s_pair_b(j // 2 - 1)
        s_pair_b(NP - 1)
        for p in range(NP):
            s_pair_c(p)         # ACT: all sigmas grouped -> one table switch
        nc.sync.dma_start(out=szf_d, in_=szf)

    nc.compile()
    return nc


_CACHE = {}


def _get_program(pix, with_bias):
    key = (pix, with_bias)
    if key not in _CACHE:
        _CACHE[key] = build_program(pix, with_bias)
    return _CACHE[key]


def _permute_cols(Wr):
    """[.., K, 10] field-blocks -> field-major columns
    [loc(c,k) | s(c,k) | cc(c,k) | logit(k)], with prescales folded:
    loc *= -127.5 (A = vp2 - 127.5*locs_m), cc *= -1 (h = q*cc')."""
    parts = []
    for c in range(C):
        parts.append(Wr[..., :, 1 + c] * -127.5)     # loc_c over k
    for c in range(C):
        parts.append(Wr[..., :, 4 + c])              # s_c over k
    for c in range(C):
        parts.append(Wr[..., :, 7 + c] * -1.0)       # cc_c over k
    parts.append(Wr[..., :, 0])                      # logit over k
    return np.concatenate(parts, axis=-1)


def _prescale_w(W_conv):
    Wr = W_conv.astype(np.float64).reshape(D, K, 10)
    Wp = _permute_cols(Wr)                           # [D, 100]
    wsb = Wp.reshape(2, 128, M).transpose(1, 0, 2).reshape(128, 2 * M)
    return np.ascontiguousarray(wsb.astype(ml_dtypes.bfloat16))


def _pack_host(vf, per):
    """vf [per,3] raw 0..255 -> (vp, q, msk) host tensors for one core.

    Pixel local index = t*2048 + s*128 + partition; partition-major:
      vp  [128, NT*C*NS]   vp2 = v-127                  at t*48 + c*16 + s
      q   [128, NT*2*160]  (vp2_j - 0.5) k-materialized at t*320 + j*160
                                                          + s*10 + k
      msk [128, NT*2*480]  PK=32768*(v==255) k-mat    at t*960 + s*30+c*10+k
                           MK=32768*(v==0)   k-mat    at t*960+480 + ...
    """
    NT = per // 2048
    v = vf.reshape(NT, NS, 128, C).transpose(2, 0, 3, 1)   # [128, t, c, s]
    vp = np.ascontiguousarray(
        (v - 127.0).reshape(128, -1).astype(np.float16))
    vs = v.transpose(0, 1, 3, 2)                           # [128, t, s, c]
    qk = np.broadcast_to((vs[:, :, None, :, 0:2] - 127.5).transpose(
        0, 1, 2, 4, 3)[..., None], (128, NT, 1, 2, NS, K)) # [128,t,1,j,s,k]
    q = np.ascontiguousarray(qk.reshape(128, -1).astype(np.float16))
    pk = KBIG * (vs == 255.0)                              # [128, t, s, c]
    mk = KBIG * (vs == 0.0)
    msk = np.stack([pk, mk], axis=2)                       # [128, t, b, s, c]
    msk = np.broadcast_to(msk[..., None], (128, NT, 2, NS, C, K))
    msk = np.ascontiguousarray(msk.reshape(128, -1).astype(np.float16))
    return vp, q, msk


def shard_inputs(x, value, W_conv, b_conv, n_cores=N_CORES):
    B = x.shape[0]
    pix_total = B * x.shape[1] * x.shape[2]
    per = pix_total // n_cores
    xf = np.ascontiguousarray(x.reshape(pix_total, D).astype(np.float32))
    vf = value.reshape(pix_total, C).astype(np.float32)
    w_bf = _prescale_w(np.asarray(W_conv))
    with_bias = bool(np.any(b_conv))
    in_maps = []
    for i in range(n_cores):
        xdt = ml_dtypes.float8_e4m3fn if X_FP8 else ml_dtypes.bfloat16
        xT = xf[i * per:(i + 1) * per].T.astype(xdt)
        # tile-contiguous layout [NT*D, 2048]: each tile DMA one dense block
        xT = np.ascontiguousarray(
            xT.reshape(D, per // 2048, 2048).transpose(1, 0, 2)
        ).reshape(-1, 2048)
        vp, q, msk = _pack_host(vf[i * per:(i + 1) * per], per)
        mm = {"xT": xT, "w": w_bf, "vp": vp, "q": q, "msk": msk}
        if with_bias:
            br = _permute_cols(
                b_conv.astype(np.float64).reshape(K, 10)[None, :, :])
            mm["bias"] = br.reshape(1, M).astype(ml_dtypes.bfloat16)
        in_maps.append(mm)
    return in_maps, with_bias, per


def _finish_host(szf, per):
    """szf [128, 2*NSL] f32 -> per-image mix_lp sums [NIMG] (f64 math)."""
    NSL = per // 128
    sz = szf[:, 0:NSL].astype(np.float64)
    sf = szf[:, NSL:2 * NSL].astype(np.float64)
    mix = np.log(np.maximum(sz, 1e-300)) - np.log(sf)
    # pixel = t*2048 + s*128 + p ; image = pixel // 1024 -> (t, s//8)
    NT = per // 2048
    return mix.reshape(128, NT, 2, 8).sum(axis=(0, 3)).reshape(-1)


def kernel(x, value, W_conv, b_conv):
    x = np.asarray(x)
    value = np.asarray(value)
    W_conv = np.asarray(W_conv)
    b_conv = np.asarray(b_conv)
    in_maps, with_bias, per = shard_inputs(x, value, W_conv, b_conv)
    nc = _get_program(per, with_bias)
    res = run_bass_kernel_spmd(nc, in_maps, list(range(N_CORES)))
    parts = [
        _finish_host(res.results[i]["szf"], per).astype(np.float32)
        for i in range(N_CORES)
    ]
    return np.concatenate(parts)
